# revision 28
# baseline (speedup 1.0000x reference)
"""GCN layer kernel for Trainium2, SPMD over 8 NeuronCores.

Reference computation (fp32):
    adj_hat = rownorm(adj + I)                      # [N, N]
    out     = adj_hat @ (X @ W) + bias              # X: [N, T, A]

Sharding: T (time) axis split across 8 cores; adj/W/bias replicated.

Numerics: fp16 end-to-end on the wire (X, W, adjacency, intermediate Y,
and the output), fp32 accumulation in PSUM. Measured rel-err of this
scheme vs the fp32 reference is 4.6e-4 (gate is 2e-2). Halving the HBM
traffic matters because the per-core kernel sits right on the ridge of
the ~358 GB/s HBM-per-NeuronCore roofline (32 MB moved -> ~90us) and
the PE roofline (196K matmul columns at 1 col/cyc fp16 -> ~82us).
(An fp8e3m4 X input was validated at rel-err 1.1e-2 but buys no time -
the stream is PE-bound once I/O is 16-bit - so fp16 is kept for margin.)

Per-core kernel (T_SH = 256 time steps):
  setup (once): load adj [m,n] fp32; r[m] = 1/(1+rowsum adj); scale
    (adj * r) -> fp16 and PE-transpose into adjT_hat [n, m] with
    diag(r) added on the diagonal blocks, so the whole row
    normalization is pre-folded into the aggregation operand and the
    hot loop has no per-t scaling. Load W [a,o] fp16 and bias [o,1].
  per t: Y_t^T[a, m] = sum_nck matmul(lhsT=X_t[n,a], rhs=adjT_hat[n,m])
         (X's natural [n, (t a)] SBUF layout is the stationary operand;
          moving operand is the constant 256-col adjT_hat -> 1 cyc/col)
  per 4 t (two PSUM banks): ONE ACT copy Y^T PSUM -> SBUF fp16
         (1024 elems - ACTIVATE costs (N+352)/1.2 ns, so big N amortizes
         the 352-cycle fixed cost), then two 512-col matmuls with the
         CONSTANT stationary W (512 fp32 = one PSUM bank per matmul):
         psum[o, 2t*m] = matmul(lhsT=W[a,o], rhs=Y^T[a, 2t*m])
         and one 1024-elem bias epilogue out_sb[o,4t,m] = psum + bias[o]
         - 8/9 of epilogues on DVE tensor_scalar, 1/9 on ACT
         (Identity+bias AP) to balance ACT at ~80us vs DVE at ~78us.
  GEMM2 + epilogue run TWO 4t-groups behind GEMM1 (software pipelining)
  so the in-order PE queue never waits on the ACT copies.
  Output is written TRANSPOSED as out[o, t, m] fp16 (contiguous runs
  per partition); the host restores [m, t, o] fp32. This lets GEMM2
  keep W stationary and stream 512-col moving operands instead of
  duplicating W into [W|W], which is what holds PE at ~320 cyc/t.
  X loads on the sync HWDGE ring (prefetched PF blocks deep), stores on
  the GPSIMD/SWDGE ring so store issue costs nothing on ACT. The first
  and last X blocks are small (4/12/16t ramps) so the first compute
  starts early and the final store's drain is short.

Measured: ~111-112us HW exec (baseline fp32 kernel: 193.6us). Fixed
overheads in that number: ~7us engine-init preamble before the first
DMA can issue, and ~7us of framework semaphore-teardown at the end.
"""

import os
import sys

import numpy as np

for _p in ("/opt/trn_rl_repo", "/root/.axon_site/_ro/trn_rl_repo"):
    if os.path.isdir(_p) and _p not in sys.path:
        sys.path.insert(0, _p)

import concourse.bass as bass
import concourse.mybir as mybir
import concourse.tile as tile
from concourse import bacc
from concourse.bass_utils import run_bass_kernel_spmd
from concourse.masks import make_identity

N_NODES = 256
N_TIMES = 2048
N_FEAT = 128
N_CORES = 8
T_SH = N_TIMES // N_CORES  # 256 time steps per core
P = 128  # partitions
NCH = N_NODES // P  # 2 node chunks

F32 = mybir.dt.float32
F16 = mybir.dt.float16

G = 4  # time steps per epilogue group (2 PSUM banks of GEMM2 output)


def _gcn_body(tc, out, x, adj, w, b, t_sh, blocks):
    nc = tc.nc
    assert sum(blocks) == t_sh and all(tb % G == 0 for tb in blocks)
    n_grp = t_sh // G

    from contextlib import ExitStack

    with ExitStack() as ctx:
        const = ctx.enter_context(tc.tile_pool(name="const", bufs=1))

        ident = const.tile([P, P], F16)
        make_identity(nc, ident)

        # adjacency is the head critical path (DMA -> rownorm -> transpose
        # gates the first aggregation matmul), so its loads go FIRST on the
        # sync ring
        adj_sb = const.tile([P, NCH, N_NODES], F32)
        nc.sync.dma_start(
            out=adj_sb, in_=adj.rearrange("(c m) n -> m c n", m=P)
        )
        a_sb = [adj_sb[:, mc, :] for mc in range(NCH)]

        # W [a, o] is the stationary operand of GEMM2:
        # psum[o, m] = sum_a W[a,o] * Y^T[a, m] - loaded as-is, fp16.
        w_sb = const.tile([P, N_FEAT], F16)
        nc.sync.dma_start(out=w_sb, in_=w)

        # bias as a per-partition scalar [o, 1] for the DVE epilogue
        bias_col = const.tile([P, 1], F32)
        bias_ap = bass.AP(tensor=b.tensor, offset=b.offset, ap=[b.ap[0], [0, 1]])
        nc.sync.dma_start(out=bias_col, in_=bias_ap)

        # adjT_hat[n, m] = (adj[m, n] + I) / deg[m], n on partitions, fp16.
        adjT = [
            const.tile([P, N_NODES], F16, name=f"adjT{c}", tag=f"adjT{c}")
            for c in range(NCH)
        ]

        # Main-loop SBUF pools are created BEFORE the setup scratch pool so
        # their addresses don't alias it - otherwise the first X-tile DMAs
        # inherit a WAR dependency on the whole adjacency-setup chain and the
        # DMA queue sits idle at kernel start.
        xp = ctx.enter_context(tc.tile_pool(name="xp", bufs=5))
        op = ctx.enter_context(tc.tile_pool(name="op", bufs=4))
        ysb = ctx.enter_context(tc.tile_pool(name="ysb", bufs=4))

        # [n, t, a] viewed as [n%128, n//128, t, a] so one DMA moves both
        # node chunks of a time block (per-partition runs stay contiguous)
        x4 = x.rearrange("(c n) t a -> n c t a", n=P)

        t_starts = [sum(blocks[:i]) for i in range(len(blocks))]

        def load_x(bi):
            t0, tb = t_starts[bi], blocks[bi]
            xtc = xp.tile([P, NCH, tb, N_FEAT], F16, name=f"x_{bi}", tag="x")
            nc.sync.dma_start(out=xtc, in_=x4[:, :, t0 : t0 + tb, :])
            return xtc

        setup = ctx.enter_context(tc.tile_pool(name="setup", bufs=1))
        # prime the ACT Identity table set during setup so the one-time
        # ~2.7us table load doesn't stall the first ACT-side epilogue
        warm = setup.tile([P, 1], F32, name="warm", tag="warm")
        nc.scalar.activation(
            warm, bias_col, mybir.ActivationFunctionType.Identity, bias=bias_col
        )

        PF = 4  # prefetch depth in blocks
        prefetched = [load_x(bi) for bi in range(min(PF, len(blocks)))]

        with tc.tile_pool(name="setup_ps", bufs=1, space="PSUM") as setup_ps:
            # r[m] = 1 / (1 + sum_n adj[m, n]) off the natural [m, n] layout
            ah16 = []
            dr16 = []
            for mc in range(NCH):
                dg = setup.tile([P, 1], F32, name=f"dg{mc}", tag=f"dg{mc}")
                nc.vector.reduce_sum(dg, a_sb[mc], axis=mybir.AxisListType.X)
                nc.vector.tensor_scalar_add(dg, dg, 1.0)
                r_m = setup.tile([P, 1], F32, name=f"r{mc}", tag=f"r{mc}")
                nc.vector.reciprocal(r_m, dg)
                # row-normalized adjacency, cast fp16 (still [m, n] layout)
                ah = setup.tile([P, N_NODES], F16, name=f"ah{mc}", tag=f"ah{mc}")
                nc.vector.tensor_scalar_mul(ah, a_sb[mc], r_m)
                ah16.append(ah)
                # diag(r) in fp16: identity row i scaled by r[i]
                dr = setup.tile([P, P], F16, name=f"dr{mc}", tag=f"dr{mc}")
                nc.vector.tensor_scalar_mul(dr, ident, r_m)
                dr16.append(dr)
            for nck in range(NCH):
                for mc in range(NCH):
                    tp = setup_ps.tile([P, P], F16, name="tp", tag="tp")
                    nc.tensor.transpose(
                        tp, ah16[mc][:, nck * P : (nck + 1) * P], ident
                    )
                    dst = adjT[nck][:, mc * P : (mc + 1) * P]
                    if mc == nck:
                        nc.vector.tensor_add(dst, tp, dr16[mc])
                    else:
                        nc.vector.tensor_copy(dst, tp)

        # each tile is 2 PSUM banks; 2+2 bufs = all 8 banks
        yps = ctx.enter_context(tc.tile_pool(name="yps", bufs=2, space="PSUM"))
        ops = ctx.enter_context(tc.tile_pool(name="ops", bufs=2, space="PSUM"))

        # group gi covers t in [gi*G, (gi+1)*G); map groups to blocks
        grp_blk = []
        for bi, tb in enumerate(blocks):
            grp_blk += [bi] * (tb // G)
        pend = {}  # gi -> (ysg, opt)
        ot_tiles = {}

        def emit_g1(gi):
            """aggregation matmuls + one 1024-elem ACT copy per group"""
            bi = grp_blk[gi]
            t0b = t_starts[bi]
            xt = prefetched[bi]
            ysg = ysb.tile([P, G, N_NODES], F16, name=f"ys{gi}", tag="ys")
            ypt4 = yps.tile([P, G, N_NODES], F32, name="ypt", tag="y")
            for tt in range(G):
                ti = gi * G + tt - t0b  # t within block
                for ck in range(NCH):
                    nc.tensor.matmul(
                        ypt4[:, tt, :],
                        xt[:, ck, ti, :],
                        adjT[ck],
                        start=(ck == 0),
                        stop=(ck == NCH - 1),
                    )
            nc.scalar.copy(ysg, ypt4)
            pend[gi] = ysg

        def emit_g2(gi):
            """512-col W matmuls + one 1024-elem bias epilogue + store"""
            ysg = pend.pop(gi)
            opt = ops.tile([P, G, N_NODES], F32, name="opt", tag="op")
            bi = grp_blk[gi]
            t0b, tb = t_starts[bi], blocks[bi]
            if gi == 0 or grp_blk[gi - 1] != bi:
                ot_tiles[bi] = op.tile(
                    [P, tb, N_NODES], F16, name=f"o_{bi}", tag="o"
                )
            ot = ot_tiles[bi]
            for h in range(G // 2):
                nc.tensor.matmul(
                    opt[:, h * 2 : (h + 1) * 2, :].rearrange("p t m -> p (t m)"),
                    w_sb,
                    ysg[:, h * 2 : (h + 1) * 2, :].rearrange("p t m -> p (t m)"),
                    start=True,
                    stop=True,
                )
            g0 = gi * G - t0b  # first t of group within block
            if gi % 9 == 8:
                # ~1/9 of the epilogues run on ACT (Identity + per-partition
                # bias) to balance ACT vs DVE occupancy
                nc.scalar.activation(
                    ot[:, g0 : g0 + G, :],
                    opt,
                    mybir.ActivationFunctionType.Identity,
                    bias=bias_col,
                )
            else:
                nc.vector.tensor_scalar_add(
                    ot[:, g0 : g0 + G, :], opt, bias_col
                )
            if gi == n_grp - 1 or grp_blk[gi + 1] != bi:
                # last group of the block: store via SWDGE (costs no ACT time)
                nc.gpsimd.dma_start(
                    out=out[:, t0b : t0b + tb, :], in_=ot_tiles.pop(bi)
                )

        # GEMM2 trails GEMM1 by two 4t-groups so its matmuls never wait on
        # the ACT copy of their ysg operand
        DELAY = 2
        for gi in range(n_grp + DELAY):
            if gi < n_grp:
                bi = grp_blk[gi]
                # block boundary: slide the X prefetch window before any
                # of this block's compute enters the queues
                if (gi == 0 or grp_blk[gi - 1] != bi) and bi + PF < len(blocks):
                    prefetched.append(load_x(bi + PF))
                emit_g1(gi)
            if gi >= DELAY:
                emit_g2(gi - DELAY)


def build(t_sh=T_SH, tb=None):
    """Build + compile the per-core Bass module."""
    if tb is None:
        # small leading blocks so the first compute starts earlier, and a
        # small trailing block so the final store's drain is short
        blocks = [4, 12, 16] + [32] * ((t_sh - 64) // 32) + [16, 12, 4]
    else:
        blocks = [tb] * (t_sh // tb)
    nc = bacc.Bacc(
        "TRN2", target_bir_lowering=False, debug=False, num_devices=N_CORES
    )
    x = nc.dram_tensor("node_feats", [N_NODES, t_sh, N_FEAT], F16, kind="ExternalInput").ap()
    adj = nc.dram_tensor("adj_matrix", [N_NODES, N_NODES], F32, kind="ExternalInput").ap()
    w = nc.dram_tensor("weight", [N_FEAT, N_FEAT], F16, kind="ExternalInput").ap()
    b = nc.dram_tensor("bias", [N_FEAT], F32, kind="ExternalInput").ap()
    # output is TRANSPOSED: [o, t, m] fp16; host restores [m, t, o] fp32
    out = nc.dram_tensor("out", [N_FEAT, t_sh, N_NODES], F16, kind="ExternalOutput").ap()
    with tile.TileContext(nc) as tc:
        _gcn_body(tc, out, x, adj, w, b, t_sh, blocks)
    nc.compile()
    return nc


_built_nc = None


def _get_nc():
    global _built_nc
    if _built_nc is None:
        _built_nc = build()
    return _built_nc


def _run(node_feats, adj_matrix, weight, bias, trace=False, tmpdir=None):
    nc = _get_nc()
    x8 = np.asarray(node_feats, dtype=np.float16)
    adj_matrix = np.ascontiguousarray(adj_matrix, dtype=np.float32)
    w16 = np.ascontiguousarray(weight, dtype=np.float16)
    bias = np.ascontiguousarray(bias, dtype=np.float32)
    in_maps = [
        {
            "node_feats": np.ascontiguousarray(
                x8[:, c * T_SH : (c + 1) * T_SH, :]
            ),
            "adj_matrix": adj_matrix,
            "weight": w16,
            "bias": bias,
        }
        for c in range(N_CORES)
    ]
    res = run_bass_kernel_spmd(
        nc, in_maps, list(range(N_CORES)), trace=trace, tmpdir=tmpdir
    )
    out = np.empty((N_NODES, N_TIMES, N_FEAT), dtype=np.float32)
    for c in range(N_CORES):
        # per-core result is [o, t, m] fp16 -> [m, t, o] fp32
        out[:, c * T_SH : (c + 1) * T_SH, :] = np.asarray(
            res.results[c]["out"], dtype=np.float32
        ).transpose(2, 1, 0)
    return out, res


def kernel(node_feats, adj_matrix, weight, bias):
    out, _ = _run(node_feats, adj_matrix, weight, bias)
    return out


# revision 29
# speedup vs baseline: 1.0331x; 1.0331x over previous
"""GCN layer kernel for Trainium2, SPMD over 8 NeuronCores.

Reference computation (fp32):
    adj_hat = rownorm(adj + I)                      # [N, N]
    out     = adj_hat @ (X @ W) + bias              # X: [N, T, A]

Sharding: T (time) axis split across 8 cores; adj/W/bias replicated.

Numerics: fp16 end-to-end on the wire (X, W, adjacency, intermediate Y,
and the output), fp32 accumulation in PSUM. Measured rel-err of this
scheme vs the fp32 reference is 4.6e-4 (gate is 2e-2). Halving the HBM
traffic matters because the per-core kernel sits right on the ridge of
the ~358 GB/s HBM-per-NeuronCore roofline (32 MB moved -> ~90us) and
the PE roofline (196K matmul columns at 1 col/cyc fp16 -> ~82us).
(An fp8e3m4 X input was validated at rel-err 1.1e-2 but buys no time -
the stream is PE-bound once I/O is 16-bit - so fp16 is kept for margin.)

Per-core kernel (T_SH = 256 time steps):
  setup (once): load adj [m,n] fp32; r[m] = 1/(1+rowsum adj); scale
    (adj * r) -> fp16 and PE-transpose into adjT_hat [n, m] with
    diag(r) added on the diagonal blocks, so the whole row
    normalization is pre-folded into the aggregation operand and the
    hot loop has no per-t scaling. Load W [a,o] fp16 and bias [o,1].
  per t: Y_t^T[a, m] = sum_nck matmul(lhsT=X_t[n,a], rhs=adjT_hat[n,m])
         (X's natural [n, (t a)] SBUF layout is the stationary operand;
          moving operand is the constant 256-col adjT_hat -> 1 cyc/col)
  per 4 t (two PSUM banks): ONE ACT copy Y^T PSUM -> SBUF fp16
         (1024 elems - ACTIVATE costs (N+352)/1.2 ns, so big N amortizes
         the 352-cycle fixed cost), then two 512-col matmuls with the
         CONSTANT stationary W (512 fp32 = one PSUM bank per matmul):
         psum[o, 2t*m] = matmul(lhsT=W[a,o], rhs=Y^T[a, 2t*m])
         and one 1024-elem bias epilogue out_sb[o,4t,m] = psum + bias[o]
         - 8/9 of epilogues on DVE tensor_scalar, 1/9 on ACT
         (Identity+bias AP) to balance ACT at ~80us vs DVE at ~78us.
  GEMM2 + epilogue run TWO 4t-groups behind GEMM1 (software pipelining)
  so the in-order PE queue never waits on the ACT copies.
  Output is written TRANSPOSED as out[o, t, m] fp16 (contiguous runs
  per partition); the host restores [m, t, o] fp32. This lets GEMM2
  keep W stationary and stream 512-col moving operands instead of
  duplicating W into [W|W], which is what holds PE at ~320 cyc/t.
  X loads on the sync HWDGE ring (prefetched PF blocks deep), stores on
  the GPSIMD/SWDGE ring so store issue costs nothing on ACT. The first
  and last X blocks are small (4/12/16t ramps) so the first compute
  starts early and the final store's drain is short.

Measured: ~111-112us HW exec (baseline fp32 kernel: 193.6us). Fixed
overheads in that number: ~7us engine-init preamble before the first
DMA can issue, and ~7us of framework semaphore-teardown at the end.
"""

import os
import sys

import numpy as np

for _p in ("/opt/trn_rl_repo", "/root/.axon_site/_ro/trn_rl_repo"):
    if os.path.isdir(_p) and _p not in sys.path:
        sys.path.insert(0, _p)

import concourse.bass as bass
import concourse.mybir as mybir
import concourse.tile as tile
from concourse import bacc
from concourse.bass_utils import run_bass_kernel_spmd
from concourse.masks import make_identity

N_NODES = 256
N_TIMES = 2048
N_FEAT = 128
N_CORES = 8
T_SH = N_TIMES // N_CORES  # 256 time steps per core
P = 128  # partitions
NCH = N_NODES // P  # 2 node chunks

F32 = mybir.dt.float32
F16 = mybir.dt.float16

G = 4  # time steps per epilogue group (2 PSUM banks of GEMM2 output)


def _gcn_body(tc, out, x, adj, w, b, t_sh, blocks):
    nc = tc.nc
    assert sum(blocks) == t_sh and all(tb % G == 0 for tb in blocks)
    n_grp = t_sh // G

    from contextlib import ExitStack

    with ExitStack() as ctx:
        const = ctx.enter_context(tc.tile_pool(name="const", bufs=1))

        ident = const.tile([P, P], F16)
        make_identity(nc, ident)

        # adjacency is the head critical path (DMA -> rownorm -> transpose
        # gates the first aggregation matmul), so its loads go FIRST on the
        # sync ring
        adj_sb = const.tile([P, NCH, N_NODES], F32)
        nc.sync.dma_start(
            out=adj_sb, in_=adj.rearrange("(c m) n -> m c n", m=P)
        )
        a_sb = [adj_sb[:, mc, :] for mc in range(NCH)]

        # W [a, o] is the stationary operand of GEMM2:
        # psum[o, m] = sum_a W[a,o] * Y^T[a, m] - loaded as-is, fp16.
        w_sb = const.tile([P, N_FEAT], F16)
        nc.sync.dma_start(out=w_sb, in_=w)

        # bias as a per-partition scalar [o, 1] for the DVE epilogue
        bias_col = const.tile([P, 1], F32)
        bias_ap = bass.AP(tensor=b.tensor, offset=b.offset, ap=[b.ap[0], [0, 1]])
        nc.sync.dma_start(out=bias_col, in_=bias_ap)

        # adjT_hat[n, m] = (adj[m, n] + I) / deg[m], n on partitions, fp16.
        adjT = [
            const.tile([P, N_NODES], F16, name=f"adjT{c}", tag=f"adjT{c}")
            for c in range(NCH)
        ]

        # Main-loop SBUF pools are created BEFORE the setup scratch pool so
        # their addresses don't alias it - otherwise the first X-tile DMAs
        # inherit a WAR dependency on the whole adjacency-setup chain and the
        # DMA queue sits idle at kernel start.
        xp = ctx.enter_context(tc.tile_pool(name="xp", bufs=5))
        op = ctx.enter_context(tc.tile_pool(name="op", bufs=4))
        ysb = ctx.enter_context(tc.tile_pool(name="ysb", bufs=4))

        # [n, t, a] viewed as [n%128, n//128, t, a] so one DMA moves both
        # node chunks of a time block (per-partition runs stay contiguous)
        x4 = x.rearrange("(c n) t a -> n c t a", n=P)

        t_starts = [sum(blocks[:i]) for i in range(len(blocks))]

        def load_x(bi):
            t0, tb = t_starts[bi], blocks[bi]
            xtc = xp.tile([P, NCH, tb, N_FEAT], F16, name=f"x_{bi}", tag="x")
            nc.sync.dma_start(out=xtc, in_=x4[:, :, t0 : t0 + tb, :])
            return xtc

        setup = ctx.enter_context(tc.tile_pool(name="setup", bufs=1))
        # prime the ACT Identity table set during setup so the one-time
        # ~2.7us table load doesn't stall the first ACT-side epilogue
        warm = setup.tile([P, 1], F32, name="warm", tag="warm")
        nc.scalar.activation(
            warm, bias_col, mybir.ActivationFunctionType.Identity, bias=bias_col
        )

        PF = 4  # prefetch depth in blocks
        prefetched = [load_x(bi) for bi in range(min(PF, len(blocks)))]

        with tc.tile_pool(name="setup_ps", bufs=1, space="PSUM") as setup_ps:
            # r[m] = 1 / (1 + sum_n adj[m, n]) off the natural [m, n] layout
            ah16 = []
            dr16 = []
            for mc in range(NCH):
                dg = setup.tile([P, 1], F32, name=f"dg{mc}", tag=f"dg{mc}")
                nc.vector.reduce_sum(dg, a_sb[mc], axis=mybir.AxisListType.X)
                nc.vector.tensor_scalar_add(dg, dg, 1.0)
                r_m = setup.tile([P, 1], F32, name=f"r{mc}", tag=f"r{mc}")
                nc.vector.reciprocal(r_m, dg)
                # row-normalized adjacency, cast fp16 (still [m, n] layout)
                ah = setup.tile([P, N_NODES], F16, name=f"ah{mc}", tag=f"ah{mc}")
                nc.vector.tensor_scalar_mul(ah, a_sb[mc], r_m)
                ah16.append(ah)
                # diag(r) in fp16: identity row i scaled by r[i]
                dr = setup.tile([P, P], F16, name=f"dr{mc}", tag=f"dr{mc}")
                nc.vector.tensor_scalar_mul(dr, ident, r_m)
                dr16.append(dr)
            for nck in range(NCH):
                for mc in range(NCH):
                    tp = setup_ps.tile([P, P], F16, name="tp", tag="tp")
                    nc.tensor.transpose(
                        tp, ah16[mc][:, nck * P : (nck + 1) * P], ident
                    )
                    dst = adjT[nck][:, mc * P : (mc + 1) * P]
                    if mc == nck:
                        nc.vector.tensor_add(dst, tp, dr16[mc])
                    else:
                        nc.vector.tensor_copy(dst, tp)

        # each tile is 2 PSUM banks; 2+2 bufs = all 8 banks
        yps = ctx.enter_context(tc.tile_pool(name="yps", bufs=2, space="PSUM"))
        ops = ctx.enter_context(tc.tile_pool(name="ops", bufs=2, space="PSUM"))

        # group gi covers t in [gi*G, (gi+1)*G); map groups to blocks
        grp_blk = []
        for bi, tb in enumerate(blocks):
            grp_blk += [bi] * (tb // G)
        pend = {}  # gi -> (ysg, opt)
        ot_tiles = {}

        def emit_g1(gi):
            """aggregation matmuls + one 1024-elem ACT copy per group"""
            bi = grp_blk[gi]
            t0b = t_starts[bi]
            xt = prefetched[bi]
            ysg = ysb.tile([P, G, N_NODES], F16, name=f"ys{gi}", tag="ys")
            ypt4 = yps.tile([P, G, N_NODES], F32, name="ypt", tag="y")
            for tt in range(G):
                ti = gi * G + tt - t0b  # t within block
                for ck in range(NCH):
                    nc.tensor.matmul(
                        ypt4[:, tt, :],
                        xt[:, ck, ti, :],
                        adjT[ck],
                        start=(ck == 0),
                        stop=(ck == NCH - 1),
                    )
            nc.scalar.copy(ysg, ypt4)
            pend[gi] = ysg

        def emit_g2(gi):
            """512-col W matmuls + one 1024-elem bias epilogue + store"""
            ysg = pend.pop(gi)
            opt = ops.tile([P, G, N_NODES], F32, name="opt", tag="op")
            bi = grp_blk[gi]
            t0b, tb = t_starts[bi], blocks[bi]
            if gi == 0 or grp_blk[gi - 1] != bi:
                ot_tiles[bi] = op.tile(
                    [P, tb, N_NODES], F16, name=f"o_{bi}", tag="o"
                )
            ot = ot_tiles[bi]
            for h in range(G // 2):
                nc.tensor.matmul(
                    opt[:, h * 2 : (h + 1) * 2, :].rearrange("p t m -> p (t m)"),
                    w_sb,
                    ysg[:, h * 2 : (h + 1) * 2, :].rearrange("p t m -> p (t m)"),
                    start=True,
                    stop=True,
                )
            g0 = gi * G - t0b  # first t of group within block
            if False:  # all epilogues on DVE: keeps ACT copies on schedule
                # ~1/9 of the epilogues run on ACT (Identity + per-partition
                # bias) to balance ACT vs DVE occupancy
                nc.scalar.activation(
                    ot[:, g0 : g0 + G, :],
                    opt,
                    mybir.ActivationFunctionType.Identity,
                    bias=bias_col,
                )
            else:
                nc.vector.tensor_scalar_add(
                    ot[:, g0 : g0 + G, :], opt, bias_col
                )
            if gi == n_grp - 1 or grp_blk[gi + 1] != bi:
                # last group of the block: store via SWDGE (costs no ACT time)
                nc.gpsimd.dma_start(
                    out=out[:, t0b : t0b + tb, :], in_=ot_tiles.pop(bi)
                )

        # GEMM2 trails GEMM1 by two 4t-groups so its matmuls never wait on
        # the ACT copy of their ysg operand
        DELAY = 2
        for gi in range(n_grp + DELAY):
            if gi < n_grp:
                bi = grp_blk[gi]
                # block boundary: slide the X prefetch window before any
                # of this block's compute enters the queues
                if (gi == 0 or grp_blk[gi - 1] != bi) and bi + PF < len(blocks):
                    prefetched.append(load_x(bi + PF))
                emit_g1(gi)
            if gi >= DELAY:
                emit_g2(gi - DELAY)


def build(t_sh=T_SH, tb=None):
    """Build + compile the per-core Bass module."""
    if tb is None:
        # small leading blocks so the first compute starts earlier, and a
        # small trailing block so the final store's drain is short
        blocks = [4, 12, 16] + [32] * ((t_sh - 64) // 32) + [16, 12, 4]
    else:
        blocks = [tb] * (t_sh // tb)
    nc = bacc.Bacc(
        "TRN2", target_bir_lowering=False, debug=False, num_devices=N_CORES
    )
    x = nc.dram_tensor("node_feats", [N_NODES, t_sh, N_FEAT], F16, kind="ExternalInput").ap()
    adj = nc.dram_tensor("adj_matrix", [N_NODES, N_NODES], F32, kind="ExternalInput").ap()
    w = nc.dram_tensor("weight", [N_FEAT, N_FEAT], F16, kind="ExternalInput").ap()
    b = nc.dram_tensor("bias", [N_FEAT], F32, kind="ExternalInput").ap()
    # output is TRANSPOSED: [o, t, m] fp16; host restores [m, t, o] fp32
    out = nc.dram_tensor("out", [N_FEAT, t_sh, N_NODES], F16, kind="ExternalOutput").ap()
    with tile.TileContext(nc) as tc:
        _gcn_body(tc, out, x, adj, w, b, t_sh, blocks)
    nc.compile()
    return nc


_built_nc = None


def _get_nc():
    global _built_nc
    if _built_nc is None:
        _built_nc = build()
    return _built_nc


def _run(node_feats, adj_matrix, weight, bias, trace=False, tmpdir=None):
    nc = _get_nc()
    x8 = np.asarray(node_feats, dtype=np.float16)
    adj_matrix = np.ascontiguousarray(adj_matrix, dtype=np.float32)
    w16 = np.ascontiguousarray(weight, dtype=np.float16)
    bias = np.ascontiguousarray(bias, dtype=np.float32)
    in_maps = [
        {
            "node_feats": np.ascontiguousarray(
                x8[:, c * T_SH : (c + 1) * T_SH, :]
            ),
            "adj_matrix": adj_matrix,
            "weight": w16,
            "bias": bias,
        }
        for c in range(N_CORES)
    ]
    res = run_bass_kernel_spmd(
        nc, in_maps, list(range(N_CORES)), trace=trace, tmpdir=tmpdir
    )
    out = np.empty((N_NODES, N_TIMES, N_FEAT), dtype=np.float32)
    for c in range(N_CORES):
        # per-core result is [o, t, m] fp16 -> [m, t, o] fp32
        out[:, c * T_SH : (c + 1) * T_SH, :] = np.asarray(
            res.results[c]["out"], dtype=np.float32
        ).transpose(2, 1, 0)
    return out, res


def kernel(node_feats, adj_matrix, weight, bias):
    out, _ = _run(node_feats, adj_matrix, weight, bias)
    return out
